# revision 7
# baseline (speedup 1.0000x reference)
"""Trainium2 Bass kernel for nn_DCLMBlock (B=4, S=2048, H=1536) on 8 NeuronCores.

Sharding: token-parallel. Core c handles batch b=c//2, sequence half h=c%2
(1024 tokens). Every core receives a causal *context* of 2048 tokens ending at
its last own token (zero-padded in front for first halves), so one SPMD
program serves all cores:
  - gate matmul + the 12 head conv chains run over the full context (head
    dilations reach back up to 6144 tokens),
  - the dilated-conv stack runs over own tokens + a 192-token halo
    (cumulative receptive field of the 6 convs = 189),
  - everything else (router, mix, proj, FFN, path gates) over own 1024 tokens.

All activations live feature-major ([feature_partition, token_free]); matmuls
run in bf16 with fp32 PSUM accumulation; depthwise dilated convs are fused
DVE scalar_tensor_tensor chains (out = x_shift*w_tap + acc) with clamped tap
ranges that reproduce exact causal zero-padding. Head-conv biases are handled
by linearity: the on-chip chain is bias-free and the host adds a precomputed
data-independent bias profile. The three path outputs are returned separately
(scaled by their sigmoid path gates) and the host does the final fp32
residual sum out = x + pc + ps + pf.
"""

import numpy as np
import ml_dtypes

import concourse.bass as bass
import concourse.tile as tile
from concourse import mybir
from concourse.bass_utils import run_bass_kernel_spmd

F32 = mybir.dt.float32
BF16 = mybir.dt.bfloat16
AF = mybir.ActivationFunctionType
OP = mybir.AluOpType
BF = ml_dtypes.bfloat16

B, S, H = 4, 2048, 1536
NH, HD = 12, 128
KK = 4
INNER = 6144
EPS = 1e-6
CONV_DILS = (1, 2, 4, 8, 16, 32)
HEAD_DILS = [(1, 2, 4), (1, 1, 1), (4, 8, 16), (8, 16, 32), (32, 64, 128),
             (64, 128, 256), (256, 512, 1024), (1, 100, 200), (1, 500, 1000),
             (1, 1024, 2048), (3, 9, 27), (5, 25, 125)]

NKH = H // 128          # 12 feature tiles
NKI = INNER // 128      # 48 inner tiles
CTX = 2048              # context tokens per core
OWN = 1024              # own tokens per core
W = 1216                # conv-stack window (192 halo + 1024 own)
WPAD = 192
N_CORES = 8


# ---------------------------------------------------------------------------
# walrus sync-wait capacity workaround (this build allows <=1 wait per inst)
# ---------------------------------------------------------------------------
def _fix_sync_capacity(nc, dummy_ap):
    ET = mybir.EngineType
    was_frozen = nc._frozen
    nc._frozen = False

    def mk_stub(engine_ty, waits, updates):
        if engine_ty == ET.SP:
            inst = nc.sync.nop(nofuse=True)
        elif engine_ty == ET.DVE:
            inst = nc.vector.tensor_copy(dummy_ap, dummy_ap)
        elif engine_ty == ET.Activation:
            inst = nc.scalar.activation(dummy_ap, dummy_ap, AF.Copy)
        elif engine_ty == ET.PE:
            inst = nc.tensor.drain()
        elif engine_ty == ET.Pool:
            inst = nc.gpsimd.tensor_copy(dummy_ap, dummy_ap)
        else:
            raise RuntimeError(f"no stub for engine {engine_ty}")
        popped = nc.cur_bb.bb.instructions.pop()
        assert popped is inst.ins
        inst.ins.sync_info = mybir.SyncInfo(
            on_wait=list(waits), on_update=list(updates)
        )
        return inst.ins

    for f in nc.m.functions:
        for bb in f.blocks:
            out = []
            changed = False
            for ins in bb.instructions:
                si = ins.sync_info
                if si is not None and len(si.on_wait) > 1:
                    for wt in si.on_wait[1:]:
                        out.append(mk_stub(ins.engine, [wt], []))
                    del si.on_wait[1:]
                    changed = True
                out.append(ins)
                if si is not None and len(si.on_update) > 1:
                    for u in si.on_update[1:]:
                        out.append(mk_stub(ins.engine, [], [u]))
                    del si.on_update[1:]
                    changed = True
            if changed:
                bb.instructions[:] = out
    nc._frozen = was_frozen


# ---------------------------------------------------------------------------
# device program
# ---------------------------------------------------------------------------
def _build():
    nc = bass.Bass()
    dummy = nc.sbuf_tensor([1, 1], F32).__enter__()

    def par(name, shape, dt):
        return nc.declare_dram_parameter(name, shape, dt, isOutput=False)

    xc_d = par("xc", [NKH, 128, CTX], BF16)
    cw_d = par("cw", [128, NKH, 6, KK], F32)
    cb_d = par("cb", [128, NKH, 6], F32)
    nw1_d = par("nw1", [128, NKH], F32)
    hww_d = par("hww", [128, NH, 3, KK], F32)
    cprof_d = par("cprof", [NH, 128, OWN], BF16)
    mask_d = par("mask", [1, W], BF16)
    oneh_d = par("oneh", [NH, NH * 128], BF16)
    wg_d = par("wg", [24, 128, NKH, 128], BF16)
    wr_d = par("wr", [128, NKH, NH], BF16)
    rb_d = par("rb", [NH, 1], F32)
    wpg_d = par("wpg", [128, NKH, 3], BF16)
    pgb_d = par("pgb", [3, 1], F32)
    wproj_d = par("wproj", [NKH, 128, NKH, 128], BF16)
    projb_d = par("projb", [128, NKH], F32)
    wmg_d = par("wmg", [NKH, 128, NKH, 128], BF16)
    mgb_d = par("mgb", [128, NKH], F32)
    wmix_d = par("wmix", [NKH, 128, NKH, 128], BF16)
    mixb_d = par("mixb", [128, NKH], F32)
    wfi_d = par("wfi", [96, 128, NKH, 128], BF16)
    wfo_d = par("wfo", [2, NKI, 128, 6, 128], BF16)

    po_c = nc.declare_dram_parameter("po_c", [NKH, 2, 128, 512], F32, isOutput=True)
    po_s = nc.declare_dram_parameter("po_s", [NKH, 2, 128, 512], F32, isOutput=True)
    po_f = nc.declare_dram_parameter("po_f", [NKH, 2, 128, 512], F32, isOutput=True)

    dma = nc.sync.dma_start

    from contextlib import ExitStack
    with tile.TileContext(nc) as tc:
        with ExitStack() as es:
            ec = es.enter_context
            const = ec(tc.tile_pool(name="const", bufs=1))
            psp = ec(tc.tile_pool(name="ps", bufs=8, space="PSUM"))
            xnp = ec(tc.tile_pool(name="xn", bufs=NKH))
            wsl = ec(tc.tile_pool(name="wsl", bufs=4))
            smalls = ec(tc.tile_pool(name="smalls", bufs=1))
            g3bp = ec(tc.tile_pool(name="g3bp", bufs=3))
            sgp = ec(tc.tile_pool(name="sgt", bufs=4))
            ftp = ec(tc.tile_pool(name="ftmp", bufs=4))
            drp = ec(tc.tile_pool(name="dram", bufs=1, space="DRAM"))

            # ---- constants ----
            t_cw = const.tile([128, NKH, 6, KK], F32)
            dma(t_cw[:], cw_d[:])
            t_cb = const.tile([128, NKH, 6], F32)
            dma(t_cb[:], cb_d[:])
            t_nw1 = const.tile([128, NKH], F32)
            dma(t_nw1[:], nw1_d[:])
            t_hww = const.tile([128, NH, 3, KK], F32)
            dma(t_hww[:], hww_d[:])
            t_mask = const.tile([1, W], BF16)
            dma(t_mask[:], mask_d[:])
            t_oneh = const.tile([NH, NH * 128], BF16)
            dma(t_oneh[:], oneh_d[:])
            t_wr = const.tile([128, NKH, NH], BF16)
            dma(t_wr[:], wr_d[:])
            t_rb = const.tile([NH, 1], F32)
            dma(t_rb[:], rb_d[:])
            t_wpg = const.tile([128, NKH, 3], BF16)
            dma(t_wpg[:], wpg_d[:])
            t_pgb = const.tile([3, 1], F32)
            dma(t_pgb[:], pgb_d[:])
            t_projb = const.tile([128, NKH], F32)
            dma(t_projb[:], projb_d[:])
            t_mgb = const.tile([128, NKH], F32)
            dma(t_mgb[:], mgb_d[:])
            t_mixb = const.tile([128, NKH], F32)
            dma(t_mixb[:], mixb_d[:])
            ones_k = const.tile([128, 1], BF16)
            nc.vector.memset(ones_k[:], 1.0)
            eps_t = const.tile([1, 1], F32)
            nc.vector.memset(eps_t[:], EPS)
            ones_b = const.tile([1, 128], BF16)
            nc.vector.memset(ones_b[:], 1.0)

            # =========== Phase R: load x, rmsnorm scale, xn ===========
            esR = ExitStack()
            xcp = esR.enter_context(tc.tile_pool(name="xc", bufs=NKH))
            x2p = esR.enter_context(tc.tile_pool(name="x2", bufs=3))
            rtp = esR.enter_context(tc.tile_pool(name="rt", bufs=1))
            rbcp = esR.enter_context(tc.tile_pool(name="rbc", bufs=1))

            xct = []
            for t in range(NKH):
                xt = xcp.tile([128, CTX], BF16, tag="xc")
                dma(xt[:], xc_d[t])
                xct.append(xt)
            sq = rtp.tile([1, CTX], F32, tag="sq")
            for c in range(4):
                cs = slice(512 * c, 512 * (c + 1))
                pss = psp.tile([1, 512], F32, tag="ps")
                for t in range(NKH):
                    x2t = x2p.tile([128, 512], BF16, tag="x2")
                    nc.vector.tensor_mul(x2t[:], xct[t][:, cs], xct[t][:, cs])
                    nc.tensor.matmul(pss[:], ones_k[:], x2t[:],
                                     start=(t == 0), stop=(t == NKH - 1))
                nc.scalar.activation(sq[:, cs], pss[:], AF.Sqrt,
                                     bias=eps_t[:], scale=1.0 / H)
            dsc = drp.tile([CTX], F32, tag="dsc")
            dma(dsc.rearrange("(p f) -> p f", p=1), sq[:])
            r128 = rtp.tile([128, CTX // 128], F32, tag="r128")
            dma(r128[:], dsc.rearrange("(p f) -> p f", p=128))
            nc.vector.reciprocal(r128[:], r128[:])
            r128b = rtp.tile([128, CTX // 128], BF16, tag="r128b")
            nc.vector.tensor_copy(r128b[:], r128[:])
            dsc2 = drp.tile([CTX], BF16, tag="dsc2")
            dma(dsc2.rearrange("(p f) -> p f", p=128), r128b[:])
            r1 = rtp.tile([1, CTX], BF16, tag="r1")
            dma(r1[:], dsc2.rearrange("(p f) -> p f", p=1))
            rb128 = rbcp.tile([128, CTX], BF16)
            for c in range(4):
                cs = slice(512 * c, 512 * (c + 1))
                pb = psp.tile([128, 512], F32, tag="ps")
                nc.tensor.matmul(pb[:], ones_b[:], r1[:, cs],
                                 start=True, stop=True)
                nc.scalar.activation(rb128[:, cs], pb[:], AF.Copy)
            xn = []
            for t in range(NKH):
                xnt = xnp.tile([128, CTX], BF16, tag="xn")
                nc.vector.tensor_mul(xnt[:], xct[t][:], rb128[:])
                xn.append(xnt)
            esR.close()

            # =========== Phase G: gate matmul (full ctx) + x_gated ===========
            xgES = ExitStack()
            xgp = xgES.enter_context(tc.tile_pool(name="xg", bufs=NH))
            xg = []
            for t in range(NH):
                wc = wsl.tile([128, NKH, 128], BF16, tag="wsl")
                dma(wc[:], wg_d[t])
                wgt = wsl.tile([128, NKH, 128], BF16, tag="wsl")
                dma(wgt[:], wg_d[t + NH])
                xgt = xgp.tile([128, CTX], BF16, tag="xg")
                for c in range(4):
                    cs = slice(512 * c, 512 * (c + 1))
                    pc = psp.tile([128, 512], F32, tag="ps")
                    for kt in range(NKH):
                        nc.tensor.matmul(pc[:], wc[:, kt, :], xn[kt][:, cs],
                                         start=(kt == 0), stop=(kt == NKH - 1))
                    pg = psp.tile([128, 512], F32, tag="ps")
                    for kt in range(NKH):
                        nc.tensor.matmul(pg[:], wgt[:, kt, :], xn[kt][:, cs],
                                         start=(kt == 0), stop=(kt == NKH - 1))
                    sg = sgp.tile([128, 512], BF16, tag="sg")
                    nc.scalar.activation(sg[:], pg[:], AF.Sigmoid)
                    nc.vector.tensor_mul(xgt[:, cs], pc[:], sg[:])
                xg.append(xgt)

            # =========== Phase P: path gates, router, mask broadcast =========
            pgs = smalls.tile([3, OWN], BF16, tag="pgs")
            for c in range(2):
                cs_own = slice(OWN + 512 * c, OWN + 512 * (c + 1))
                pp = psp.tile([3, 512], F32, tag="ps")
                for kt in range(NKH):
                    nc.tensor.matmul(pp[:], t_wpg[:, kt, :], xn[kt][:, cs_own],
                                     start=(kt == 0), stop=(kt == NKH - 1))
                nc.scalar.activation(pgs[:, 512 * c:512 * (c + 1)], pp[:],
                                     AF.Sigmoid, bias=t_pgb[:, 0:1])
            g3b = []
            for i in range(3):
                gt = g3bp.tile([128, OWN], BF16, tag="g3b")
                for c in range(2):
                    cs = slice(512 * c, 512 * (c + 1))
                    pb = psp.tile([128, 512], F32, tag="ps")
                    nc.tensor.matmul(pb[:], t_oneh[0:3, 128 * i:128 * (i + 1)],
                                     pgs[:, cs], start=True, stop=True)
                    nc.scalar.activation(gt[:, cs], pb[:], AF.Copy)
                g3b.append(gt)
            gcb, gsb, gfb = g3b
            hw_sig = smalls.tile([NH, OWN], BF16, tag="hw_sig")
            for c in range(2):
                cs_own = slice(OWN + 512 * c, OWN + 512 * (c + 1))
                pr = psp.tile([NH, 512], F32, tag="ps")
                for kt in range(NKH):
                    nc.tensor.matmul(pr[:], t_wr[:, kt, :], xn[kt][:, cs_own],
                                     start=(kt == 0), stop=(kt == NKH - 1))
                nc.scalar.activation(hw_sig[:, 512 * c:512 * (c + 1)], pr[:],
                                     AF.Sigmoid, bias=t_rb[:, 0:1])
            maskb = smalls.tile([128, W], BF16, tag="maskb")
            for c0, cn in ((0, 512), (512, 512), (1024, 192)):
                pb = psp.tile([128, 512], F32, tag="ps")
                nc.tensor.matmul(pb[:, :cn], ones_b[:], t_mask[:, c0:c0 + cn],
                                 start=True, stop=True)
                nc.scalar.activation(maskb[:, c0:c0 + cn], pb[:, :cn], AF.Copy)

            # conv-stack input tiles + interleaved conv-unit emitter
            hstES = ExitStack()
            hstp = hstES.enter_context(tc.tile_pool(name="hst", bufs=NKH))
            caccES = ExitStack()
            caccp = caccES.enter_context(tc.tile_pool(name="cacc", bufs=3))
            cbiasp = caccES.enter_context(tc.tile_pool(name="cbias", bufs=3))
            cgelp = caccES.enter_context(tc.tile_pool(name="cgel", bufs=2))
            hs = []
            for t in range(NKH):
                ht = hstp.tile([128, W], BF16, tag="h")
                nc.vector.tensor_scalar_mul(ht[:], xn[t][:, CTX - W:],
                                            t_nw1[:, t:t + 1])
                hs.append(ht)

            conv_units = [(t, j) for t in range(NKH) for j in range(6)]
            conv_iter = iter(conv_units)

            def emit_conv_unit():
                try:
                    t, j = next(conv_iter)
                except StopIteration:
                    return
                d = CONV_DILS[j]
                bt = cbiasp.tile([128, W], BF16, tag="cbias")
                nc.vector.tensor_scalar_mul(bt[:], maskb[:], t_cb[:, t, j:j + 1])
                acc = caccp.tile([128, W], BF16, tag="cacc")
                nc.vector.scalar_tensor_tensor(acc[:], hs[t][:],
                                               t_cw[:, t, j, 3:4], bt[:],
                                               OP.mult, OP.add)
                for k in (2, 1, 0):
                    delta = (3 - k) * d
                    nc.vector.scalar_tensor_tensor(
                        acc[:, delta:], hs[t][:, :W - delta],
                        t_cw[:, t, j, k:k + 1], acc[:, delta:],
                        OP.mult, OP.add)
                ge = cgelp.tile([128, W], BF16, tag="cgel")
                nc.scalar.activation(ge[:], acc[:], AF.Gelu)
                nc.vector.tensor_add(hs[t][:], hs[t][:], ge[:])

            # =========== Phase F: GLU FFN (own tokens) ===========
            ffnES = ExitStack()
            hfop = ffnES.enter_context(tc.tile_pool(name="hfo", bufs=4))
            wfopp = ffnES.enter_context(tc.tile_pool(name="wfop", bufs=3))
            hrpp = ffnES.enter_context(tc.tile_pool(name="hrp", bufs=4))

            hffd = drp.tile([2, NKI, 128, 512], BF16, tag="hffd")
            for p in range(NKI):
                wcs = wsl.tile([128, NKH, 128], BF16, tag="wsl")
                dma(wcs[:], wfi_d[p])
                wgs = wsl.tile([128, NKH, 128], BF16, tag="wsl")
                dma(wgs[:], wfi_d[p + NKI])
                for c in range(2):
                    cs_own = slice(OWN + 512 * c, OWN + 512 * (c + 1))
                    pc = psp.tile([128, 512], F32, tag="ps")
                    for kt in range(NKH):
                        nc.tensor.matmul(pc[:], wcs[:, kt, :], xn[kt][:, cs_own],
                                         start=(kt == 0), stop=(kt == NKH - 1))
                    pg = psp.tile([128, 512], F32, tag="ps")
                    for kt in range(NKH):
                        nc.tensor.matmul(pg[:], wgs[:, kt, :], xn[kt][:, cs_own],
                                         start=(kt == 0), stop=(kt == NKH - 1))
                    sg = sgp.tile([128, 512], BF16, tag="sg")
                    nc.scalar.activation(sg[:], pg[:], AF.Sigmoid)
                    hoc = hfop.tile([128, 512], BF16, tag="hfo")
                    nc.vector.tensor_mul(hoc[:], pc[:], sg[:])
                    dma(hffd[c, p], hoc[:])
                emit_conv_unit()
                if p % 2 == 0:
                    emit_conv_unit()
            for _ in range(12):
                emit_conv_unit()

            # ffn_out: 2 groups of 6 output tiles, streaming hff + weights
            for g in range(2):
                for c in range(2):
                    pss = []
                    for _j in range(6):
                        psj = psp.tile([128, 512], F32, tag="ps")
                        pss.append(psj)
                    for kt in range(NKI):
                        ws = wfopp.tile([128, 6, 128], BF16, tag="wfop")
                        dma(ws[:], wfo_d[g, kt])
                        hr = hrpp.tile([128, 512], BF16, tag="hr")
                        dma(hr[:], hffd[c, kt])
                        for j in range(6):
                            nc.tensor.matmul(pss[j][:], ws[:, j, :], hr[:],
                                             start=(kt == 0), stop=(kt == NKI - 1),
                                             skip_group_check=True)
                    for j in range(6):
                        tf = ftp.tile([128, 512], F32, tag="ftmp")
                        nc.vector.tensor_mul(tf[:], pss[j][:],
                                             gfb[:, 512 * c:512 * (c + 1)])
                        dma(po_f[g * 6 + j, c], tf[:])
            ffnES.close()
            caccES.close()

            # =========== Phase H: head conv chains (full ctx, in-place) ======
            headES = ExitStack()
            haccp = headES.enter_context(tc.tile_pool(name="hacc", bufs=2))
            cpfp = headES.enter_context(tc.tile_pool(name="cpf", bufs=2))
            for i in range(NH):
                for j in range(3):
                    d = HEAD_DILS[i][j]
                    acc = haccp.tile([128, CTX], BF16, tag="hacc")
                    nc.vector.tensor_scalar_mul(acc[:], xg[i][:],
                                                t_hww[:, i, j, 3:4])
                    for k in (2, 1, 0):
                        delta = (3 - k) * d
                        if delta < CTX:
                            nc.vector.scalar_tensor_tensor(
                                acc[:, delta:], xg[i][:, :CTX - delta],
                                t_hww[:, i, j, k:k + 1], acc[:, delta:],
                                OP.mult, OP.add)
                    nc.vector.tensor_add(xg[i][:], xg[i][:], acc[:])
                cp = cpfp.tile([128, OWN], BF16, tag="cpf")
                dma(cp[:], cprof_d[i])
                nc.vector.tensor_add(xg[i][:, OWN:], xg[i][:, OWN:], cp[:])
                for c in range(2):
                    cs = slice(512 * c, 512 * (c + 1))
                    pb = psp.tile([128, 512], F32, tag="ps")
                    nc.tensor.matmul(pb[:], t_oneh[:, 128 * i:128 * (i + 1)],
                                     hw_sig[:, cs], start=True, stop=True)
                    hb = sgp.tile([128, 512], BF16, tag="sg")
                    nc.scalar.activation(hb[:], pb[:], AF.Copy)
                    cso = slice(OWN + 512 * c, OWN + 512 * (c + 1))
                    nc.vector.tensor_mul(xg[i][:, cso], xg[i][:, cso], hb[:])
            headES.close()

            # =========== conv-stack projection ===========
            for t in range(NKH):
                wp = wsl.tile([128, NKH, 128], BF16, tag="wsl")
                dma(wp[:], wproj_d[t])
                for c in range(2):
                    ws_ = slice(WPAD + 512 * c, WPAD + 512 * (c + 1))
                    pp = psp.tile([128, 512], F32, tag="ps")
                    for kt in range(NKH):
                        nc.tensor.matmul(pp[:], wp[:, kt, :], hs[kt][:, ws_],
                                         start=(kt == 0), stop=(kt == NKH - 1))
                    tb = sgp.tile([128, 512], BF16, tag="sg")
                    nc.scalar.activation(tb[:], pp[:], AF.Identity,
                                         bias=t_projb[:, t:t + 1])
                    tf = ftp.tile([128, 512], F32, tag="ftmp")
                    nc.vector.tensor_mul(tf[:], tb[:],
                                         gcb[:, 512 * c:512 * (c + 1)])
                    dma(po_c[t, c], tf[:])
            hstES.close()

            # =========== Phase M: mix gate + mixing ===========
            sgmES = ExitStack()
            sgmp = sgmES.enter_context(tc.tile_pool(name="sgm", bufs=NH))
            sgm = []
            for t in range(NKH):
                wm = wsl.tile([128, NKH, 128], BF16, tag="wsl")
                dma(wm[:], wmg_d[t])
                st = sgmp.tile([128, OWN], BF16, tag="sgm")
                for c in range(2):
                    cs_own = slice(OWN + 512 * c, OWN + 512 * (c + 1))
                    pm = psp.tile([128, 512], F32, tag="ps")
                    for kt in range(NKH):
                        nc.tensor.matmul(pm[:], wm[:, kt, :],
                                         xg[kt][:, cs_own],
                                         start=(kt == 0), stop=(kt == NKH - 1))
                    nc.scalar.activation(st[:, 512 * c:512 * (c + 1)], pm[:],
                                         AF.Sigmoid, bias=t_mgb[:, t:t + 1])
                sgm.append(st)
            for t in range(NKH):
                nc.vector.tensor_mul(xg[t][:, OWN:], xg[t][:, OWN:], sgm[t][:])
            for t in range(NKH):
                wx = wsl.tile([128, NKH, 128], BF16, tag="wsl")
                dma(wx[:], wmix_d[t])
                for c in range(2):
                    cs_own = slice(OWN + 512 * c, OWN + 512 * (c + 1))
                    pm = psp.tile([128, 512], F32, tag="ps")
                    for kt in range(NKH):
                        nc.tensor.matmul(pm[:], wx[:, kt, :],
                                         xg[kt][:, cs_own],
                                         start=(kt == 0), stop=(kt == NKH - 1))
                    tb = sgp.tile([128, 512], BF16, tag="sg")
                    nc.scalar.activation(tb[:], pm[:], AF.Identity,
                                         bias=t_mixb[:, t:t + 1])
                    tf = ftp.tile([128, 512], F32, tag="ftmp")
                    nc.vector.tensor_mul(tf[:], tb[:],
                                         gsb[:, 512 * c:512 * (c + 1)])
                    dma(po_s[t, c], tf[:])
            sgmES.close()
            xgES.close()

    nc.finalize()
    _fix_sync_capacity(nc, dummy[:])
    return nc


# ---------------------------------------------------------------------------
# host side
# ---------------------------------------------------------------------------
def _wslab(Wt, nk, no):
    """[IN, OUT] weight (already transposed to in-major) -> [no, 128, nk, 128]
    slab layout: slab[ot][p, kt, m] = Wt[kt*128+p, ot*128+m]."""
    return np.ascontiguousarray(
        Wt.reshape(nk, 128, no, 128).transpose(2, 1, 0, 3)
    ).astype(BF)


def _head_bias_profile(head_ws, head_bs):
    """Data-independent bias part of each head's (linear) conv chain over the
    global sequence, with exact causal zero padding."""
    C = np.zeros((NH, HD, S), np.float32)
    for i in range(NH):
        v = np.zeros((HD, S), np.float32)
        for j, d in enumerate(HEAD_DILS[i]):
            conv = np.zeros_like(v)
            for k in range(KK):
                delta = (3 - k) * d
                if delta == 0:
                    conv += head_ws[i, j, :, 0, k][:, None] * v
                elif delta < S:
                    conv[:, delta:] += head_ws[i, j, :, 0, k][:, None] * v[:, :-delta]
            v = v + conv + head_bs[i, j][:, None]
        C[i] = v
    return C


_NC_CACHE = {}


def kernel(**inputs):
    x = np.asarray(inputs["x"], np.float32)
    nw = np.asarray(inputs["norm_w"], np.float32)
    conv_ws = np.asarray(inputs["conv_ws"], np.float32)
    conv_bs = np.asarray(inputs["conv_bs"], np.float32)
    conv_proj_w = np.asarray(inputs["conv_proj_w"], np.float32)
    conv_proj_b = np.asarray(inputs["conv_proj_b"], np.float32)
    gate_w = np.asarray(inputs["gate_w"], np.float32)
    router_w = np.asarray(inputs["router_w"], np.float32)
    router_b = np.asarray(inputs["router_b"], np.float32)
    head_ws = np.asarray(inputs["head_ws"], np.float32)
    head_bs = np.asarray(inputs["head_bs"], np.float32)
    mix_gate_w = np.asarray(inputs["mix_gate_w"], np.float32)
    mix_gate_b = np.asarray(inputs["mix_gate_b"], np.float32)
    mixing_w = np.asarray(inputs["mixing_w"], np.float32)
    mixing_b = np.asarray(inputs["mixing_b"], np.float32)
    ffn_in_w = np.asarray(inputs["ffn_in_w"], np.float32)
    ffn_out_w = np.asarray(inputs["ffn_out_w"], np.float32)
    pg_w = np.asarray(inputs["pg_w"], np.float32)
    pg_b = np.asarray(inputs["pg_b"], np.float32)

    shared = {
        "cw": np.ascontiguousarray(
            conv_ws[:, :, 0, :].reshape(6, NKH, 128, KK).transpose(2, 1, 0, 3)),
        "cb": np.ascontiguousarray(
            conv_bs.reshape(6, NKH, 128).transpose(2, 1, 0)),
        "nw1": np.ascontiguousarray(nw[0].reshape(NKH, 128).T),
        "hww": np.ascontiguousarray(
            head_ws[:, :, :, 0, :].transpose(2, 0, 1, 3)),
        "wg": _wslab((gate_w * nw[1][None, :]).T, NKH, 24),
        "wr": np.ascontiguousarray(
            (router_w * nw[1][None, :]).T.reshape(NKH, 128, NH)
            .transpose(1, 0, 2)).astype(BF),
        "rb": router_b[:, None].astype(np.float32),
        "wpg": np.ascontiguousarray(
            (pg_w * nw).T.reshape(NKH, 128, 3).transpose(1, 0, 2)).astype(BF),
        "pgb": pg_b[:, None].astype(np.float32),
        "wproj": _wslab(conv_proj_w.T, NKH, NKH),
        "projb": np.ascontiguousarray(conv_proj_b.reshape(NKH, 128).T),
        "wmg": _wslab(mix_gate_w.T, NKH, NKH),
        "mgb": np.ascontiguousarray(mix_gate_b.reshape(NKH, 128).T),
        "wmix": _wslab(mixing_w.T, NKH, NKH),
        "mixb": np.ascontiguousarray(mixing_b.reshape(NKH, 128).T),
        "wfi": _wslab((ffn_in_w * nw[2][None, :]).T, NKH, 96),
        "wfo": np.ascontiguousarray(
            ffn_out_w.T.reshape(NKI, 128, 2, 6, 128).transpose(2, 0, 1, 3, 4)
        ).astype(BF),
    }
    oneh = np.zeros((NH, NH * 128), np.float32)
    for i in range(NH):
        oneh[i, 128 * i:128 * (i + 1)] = 1.0
    shared["oneh"] = oneh.astype(BF)

    cprof = _head_bias_profile(head_ws, head_bs)  # [NH, HD, S]
    cprof_h = [
        np.ascontiguousarray(cprof[:, :, h * OWN:(h + 1) * OWN]).astype(BF)
        for h in range(2)
    ]
    mask_h = []
    m0 = np.zeros((1, W), np.float32)
    m0[:, WPAD:] = 1.0
    mask_h.append(m0.astype(BF))
    mask_h.append(np.ones((1, W), BF))

    in_maps = []
    for core in range(N_CORES):
        b, h = core // 2, core % 2
        if h == 0:
            ctx = np.concatenate(
                [np.zeros((OWN, H), np.float32), x[b, :OWN]], axis=0)
        else:
            ctx = x[b]
        xc = np.ascontiguousarray(ctx.T.reshape(NKH, 128, CTX)).astype(BF)
        m = dict(shared)
        m["xc"] = xc
        m["cprof"] = cprof_h[h]
        m["mask"] = mask_h[h]
        in_maps.append(m)

    key = "nc"
    if key not in _NC_CACHE:
        _NC_CACHE[key] = _build()
    nc = _NC_CACHE[key]

    import os
    trace = bool(os.environ.get("BASS_KERNEL_TRACE"))
    r = run_bass_kernel_spmd(nc, in_maps, list(range(N_CORES)), trace=trace)
    global LAST_EXEC_NS
    LAST_EXEC_NS = r.exec_time_ns
    res = r.results

    out = np.empty((B, S, H), np.float32)
    for core in range(N_CORES):
        b, h = core // 2, core % 2
        total = np.zeros((H, OWN), np.float32)
        for name in ("po_c", "po_s", "po_f"):
            arr = np.asarray(res[core][name], np.float32)  # [NKH, 2, 128, 512]
            total += arr.transpose(0, 2, 1, 3).reshape(H, OWN)
        rows = slice(h * OWN, (h + 1) * OWN)
        out[b, rows, :] = x[b, rows, :] + total.T
    return out


# revision 12
# speedup vs baseline: 1.1213x; 1.1213x over previous
"""Trainium2 Bass kernel for nn_DCLMBlock (B=4, S=2048, H=1536) on 8 NeuronCores.

Sharding: token-parallel. Core c handles batch b=c//2, sequence half h=c%2
(1024 tokens). Every core receives a causal *context* of 2048 tokens ending at
its last own token (zero-padded in front for first halves), so one SPMD
program serves all cores:
  - gate matmul + the 12 head conv chains run over the full context (head
    dilations reach back up to 6144 tokens),
  - the dilated-conv stack runs over own tokens + a 192-token halo
    (cumulative receptive field of the 6 convs = 189),
  - everything else (router, mix, proj, FFN, path gates) over own 1024 tokens.

All activations live feature-major ([feature_partition, token_free]); matmuls
run in bf16 with fp32 PSUM accumulation; depthwise dilated convs are fused
DVE scalar_tensor_tensor chains (out = x_shift*w_tap + acc) with clamped tap
ranges that reproduce exact causal zero-padding. Head-conv biases are handled
by linearity: the on-chip chain is bias-free and the host adds a precomputed
data-independent bias profile. The three path outputs are returned separately
(scaled by their sigmoid path gates) and the host does the final fp32
residual sum out = x + pc + ps + pf.
"""

import numpy as np
import ml_dtypes

import concourse.bass as bass
import concourse.tile as tile
from concourse import mybir
from concourse.bass_utils import run_bass_kernel_spmd

F32 = mybir.dt.float32
BF16 = mybir.dt.bfloat16
AF = mybir.ActivationFunctionType
OP = mybir.AluOpType
BF = ml_dtypes.bfloat16

B, S, H = 4, 2048, 1536
NH, HD = 12, 128
KK = 4
INNER = 6144
EPS = 1e-6
CONV_DILS = (1, 2, 4, 8, 16, 32)
HEAD_DILS = [(1, 2, 4), (1, 1, 1), (4, 8, 16), (8, 16, 32), (32, 64, 128),
             (64, 128, 256), (256, 512, 1024), (1, 100, 200), (1, 500, 1000),
             (1, 1024, 2048), (3, 9, 27), (5, 25, 125)]

HEAD_HALO = [3 * sum(ds) for ds in HEAD_DILS]
NKH = H // 128          # 12 feature tiles
NKI = INNER // 128      # 48 inner tiles
CTX = 2048              # context tokens per core
OWN = 1024              # own tokens per core
W = 1216                # conv-stack window (192 halo + 1024 own)
HEAD_WIN = [min(2048, (1024 + h + 1) // 2 * 2) for h in HEAD_HALO]
HEAD_CSTART = [max(0, (2048 - w) // 512) for w in HEAD_WIN]
WPAD = 192
N_CORES = 8


# ---------------------------------------------------------------------------
# walrus sync-wait capacity workaround (this build allows <=1 wait per inst)
# ---------------------------------------------------------------------------
def _fix_sync_capacity(nc, dummy_ap):
    ET = mybir.EngineType
    was_frozen = nc._frozen
    nc._frozen = False

    def mk_stub(engine_ty, waits, updates):
        if engine_ty == ET.SP:
            inst = nc.sync.nop(nofuse=True)
        elif engine_ty == ET.DVE:
            inst = nc.vector.tensor_copy(dummy_ap, dummy_ap)
        elif engine_ty == ET.Activation:
            inst = nc.scalar.activation(dummy_ap, dummy_ap, AF.Copy)
        elif engine_ty == ET.PE:
            inst = nc.tensor.drain()
        elif engine_ty == ET.Pool:
            inst = nc.gpsimd.tensor_copy(dummy_ap, dummy_ap)
        else:
            raise RuntimeError(f"no stub for engine {engine_ty}")
        popped = nc.cur_bb.bb.instructions.pop()
        assert popped is inst.ins
        inst.ins.sync_info = mybir.SyncInfo(
            on_wait=list(waits), on_update=list(updates)
        )
        return inst.ins

    for f in nc.m.functions:
        for bb in f.blocks:
            out = []
            changed = False
            for ins in bb.instructions:
                si = ins.sync_info
                if si is not None and len(si.on_wait) > 1:
                    for wt in si.on_wait[1:]:
                        out.append(mk_stub(ins.engine, [wt], []))
                    del si.on_wait[1:]
                    changed = True
                out.append(ins)
                if si is not None and len(si.on_update) > 1:
                    for u in si.on_update[1:]:
                        out.append(mk_stub(ins.engine, [], [u]))
                    del si.on_update[1:]
                    changed = True
            if changed:
                bb.instructions[:] = out
    nc._frozen = was_frozen


# ---------------------------------------------------------------------------
# device program
# ---------------------------------------------------------------------------
def _build():
    nc = bass.Bass()
    dummy = nc.sbuf_tensor([1, 1], F32).__enter__()

    def par(name, shape, dt):
        return nc.declare_dram_parameter(name, shape, dt, isOutput=False)

    xc_d = par("xc", [NKH, 128, CTX], BF16)
    cw_d = par("cw", [128, NKH, 6, KK], F32)
    cb_d = par("cb", [128, NKH, 6], F32)
    nw1_d = par("nw1", [128, NKH], F32)
    hww_d = par("hww", [128, NH, 3, KK], F32)
    cprof_d = par("cprof", [NH, 128, OWN], BF16)
    mask_d = par("mask", [1, W], BF16)
    oneh_d = par("oneh", [NH, NH * 128], BF16)
    wg_d = par("wg", [24, 128, NKH, 128], BF16)
    wr_d = par("wr", [128, NKH, NH], BF16)
    rb_d = par("rb", [NH, 1], F32)
    wpg_d = par("wpg", [128, NKH, 3], BF16)
    pgb_d = par("pgb", [3, 1], F32)
    wproj_d = par("wproj", [NKH, 128, NKH, 128], BF16)
    projb_d = par("projb", [128, NKH], F32)
    wmg_d = par("wmg", [NKH, 128, NKH, 128], BF16)
    mgb_d = par("mgb", [128, NKH], F32)
    wmix_d = par("wmix", [NKH, 128, NKH, 128], BF16)
    mixb_d = par("mixb", [128, NKH], F32)
    wfi_d = par("wfi", [96, 128, NKH, 128], BF16)
    wfo_d = par("wfo", [2, NKI, 128, 6, 128], BF16)

    po_c = nc.declare_dram_parameter("po_c", [NKH, 2, 128, 512], BF16, isOutput=True)
    po_s = nc.declare_dram_parameter("po_s", [NKH, 2, 128, 512], BF16, isOutput=True)
    po_f = nc.declare_dram_parameter("po_f", [NKH, 2, 128, 512], BF16, isOutput=True)

    dma = nc.sync.dma_start

    from contextlib import ExitStack
    with tile.TileContext(nc) as tc:
        with ExitStack() as es:
            ec = es.enter_context
            const = ec(tc.tile_pool(name="const", bufs=1))
            psp = ec(tc.tile_pool(name="ps", bufs=8, space="PSUM"))
            xnp = ec(tc.tile_pool(name="xn", bufs=NKH))
            wsl = ec(tc.tile_pool(name="wsl", bufs=4))
            smalls = ec(tc.tile_pool(name="smalls", bufs=1))
            g3bp = ec(tc.tile_pool(name="g3bp", bufs=3))
            sgp = ec(tc.tile_pool(name="sgt", bufs=3))
            ftp = ec(tc.tile_pool(name="ftmp", bufs=4))
            drp = ec(tc.tile_pool(name="dram", bufs=1, space="DRAM"))

            # ---- constants ----
            t_cw = const.tile([128, NKH, 6, KK], F32)
            dma(t_cw[:], cw_d[:])
            t_cb = const.tile([128, NKH, 6], F32)
            dma(t_cb[:], cb_d[:])
            t_nw1 = const.tile([128, NKH], F32)
            dma(t_nw1[:], nw1_d[:])
            t_hww = const.tile([128, NH, 3, KK], F32)
            dma(t_hww[:], hww_d[:])
            t_mask = const.tile([1, W], BF16)
            dma(t_mask[:], mask_d[:])
            t_oneh = const.tile([NH, NH * 128], BF16)
            dma(t_oneh[:], oneh_d[:])
            t_wr = const.tile([128, NKH, NH], BF16)
            dma(t_wr[:], wr_d[:])
            t_rb = const.tile([NH, 1], F32)
            dma(t_rb[:], rb_d[:])
            t_wpg = const.tile([128, NKH, 3], BF16)
            dma(t_wpg[:], wpg_d[:])
            t_pgb = const.tile([3, 1], F32)
            dma(t_pgb[:], pgb_d[:])
            t_projb = const.tile([128, NKH], F32)
            dma(t_projb[:], projb_d[:])
            t_mgb = const.tile([128, NKH], F32)
            dma(t_mgb[:], mgb_d[:])
            t_mixb = const.tile([128, NKH], F32)
            dma(t_mixb[:], mixb_d[:])
            ones_k = const.tile([128, 1], BF16)
            nc.vector.memset(ones_k[:], 1.0)
            eps_t = const.tile([1, 1], F32)
            nc.vector.memset(eps_t[:], EPS)
            ones_b = const.tile([1, 128], BF16)
            nc.vector.memset(ones_b[:], 1.0)

            # =========== Phase R: load x, rmsnorm scale, xn ===========
            esR = ExitStack()
            xcp = esR.enter_context(tc.tile_pool(name="xc", bufs=NKH))
            x2p = esR.enter_context(tc.tile_pool(name="x2", bufs=3))
            rtp = esR.enter_context(tc.tile_pool(name="rt", bufs=1))
            rbcp = esR.enter_context(tc.tile_pool(name="rbc", bufs=1))

            xct = []
            for t in range(NKH):
                xt = xcp.tile([128, CTX], BF16, tag="xc")
                dma(xt[:], xc_d[t])
                xct.append(xt)
            sq = rtp.tile([1, CTX], F32, tag="sq")
            for c in range(4):
                cs = slice(512 * c, 512 * (c + 1))
                pss = psp.tile([1, 512], F32, tag="ps")
                for t in range(NKH):
                    x2t = x2p.tile([128, 512], BF16, tag="x2")
                    nc.vector.tensor_mul(x2t[:], xct[t][:, cs], xct[t][:, cs])
                    nc.tensor.matmul(pss[:], ones_k[:], x2t[:],
                                     start=(t == 0), stop=(t == NKH - 1))
                nc.scalar.activation(sq[:, cs], pss[:], AF.Sqrt,
                                     bias=eps_t[:], scale=1.0 / H)
            dsc = drp.tile([CTX], F32, tag="dsc")
            dma(dsc.rearrange("(p f) -> p f", p=1), sq[:])
            r128 = rtp.tile([128, CTX // 128], F32, tag="r128")
            dma(r128[:], dsc.rearrange("(p f) -> p f", p=128))
            nc.vector.reciprocal(r128[:], r128[:])
            r128b = rtp.tile([128, CTX // 128], BF16, tag="r128b")
            nc.vector.tensor_copy(r128b[:], r128[:])
            dsc2 = drp.tile([CTX], BF16, tag="dsc2")
            dma(dsc2.rearrange("(p f) -> p f", p=128), r128b[:])
            r1 = rtp.tile([1, CTX], BF16, tag="r1")
            dma(r1[:], dsc2.rearrange("(p f) -> p f", p=1))
            rb128 = rbcp.tile([128, CTX], BF16)
            for c in range(4):
                cs = slice(512 * c, 512 * (c + 1))
                pb = psp.tile([128, 512], F32, tag="ps")
                nc.tensor.matmul(pb[:], ones_b[:], r1[:, cs],
                                 start=True, stop=True)
                nc.scalar.activation(rb128[:, cs], pb[:], AF.Copy)
            xn = []
            for t in range(NKH):
                xnt = xnp.tile([128, CTX], BF16, tag="xn")
                nc.vector.tensor_mul(xnt[:], xct[t][:], rb128[:])
                xn.append(xnt)
            esR.close()

            # =========== Phase G: gate matmul (full ctx) + x_gated ===========
            xgES = ExitStack()
            xgp = xgES.enter_context(tc.tile_pool(name="xg", bufs=NH))
            xg = []
            for t in range(NH):
                wc = wsl.tile([128, NKH, 128], BF16, tag="wsl")
                dma(wc[:], wg_d[t])
                wgt = wsl.tile([128, NKH, 128], BF16, tag="wsl")
                dma(wgt[:], wg_d[t + NH])
                xgt = xgp.tile([128, CTX], BF16, tag="xg")
                for c in range(HEAD_CSTART[t], 4):
                    cs = slice(512 * c, 512 * (c + 1))
                    pc = psp.tile([128, 512], F32, tag="ps")
                    for kt in range(NKH):
                        nc.tensor.matmul(pc[:], wc[:, kt, :], xn[kt][:, cs],
                                         start=(kt == 0), stop=(kt == NKH - 1))
                    pg = psp.tile([128, 512], F32, tag="ps")
                    for kt in range(NKH):
                        nc.tensor.matmul(pg[:], wgt[:, kt, :], xn[kt][:, cs],
                                         start=(kt == 0), stop=(kt == NKH - 1))
                    pcb = sgp.tile([128, 512], BF16, tag="pcb")
                    nc.scalar.activation(pcb[:], pc[:], AF.Copy)
                    sg = sgp.tile([128, 512], BF16, tag="sg")
                    nc.scalar.activation(sg[:], pg[:], AF.Sigmoid)
                    nc.vector.tensor_mul(xgt[:, cs], pcb[:], sg[:])
                xg.append(xgt)

            # =========== Phase P: path gates, router, mask broadcast =========
            pgs = smalls.tile([3, OWN], BF16, tag="pgs")
            for c in range(2):
                cs_own = slice(OWN + 512 * c, OWN + 512 * (c + 1))
                pp = psp.tile([3, 512], F32, tag="ps")
                for kt in range(NKH):
                    nc.tensor.matmul(pp[:], t_wpg[:, kt, :], xn[kt][:, cs_own],
                                     start=(kt == 0), stop=(kt == NKH - 1))
                nc.scalar.activation(pgs[:, 512 * c:512 * (c + 1)], pp[:],
                                     AF.Sigmoid, bias=t_pgb[:, 0:1])
            g3b = []
            for i in range(3):
                gt = g3bp.tile([128, OWN], BF16, tag="g3b")
                for c in range(2):
                    cs = slice(512 * c, 512 * (c + 1))
                    pb = psp.tile([128, 512], F32, tag="ps")
                    nc.tensor.matmul(pb[:], t_oneh[0:3, 128 * i:128 * (i + 1)],
                                     pgs[:, cs], start=True, stop=True)
                    nc.scalar.activation(gt[:, cs], pb[:], AF.Copy)
                g3b.append(gt)
            gcb, gsb, gfb = g3b
            hw_sig = smalls.tile([NH, OWN], BF16, tag="hw_sig")
            for c in range(2):
                cs_own = slice(OWN + 512 * c, OWN + 512 * (c + 1))
                pr = psp.tile([NH, 512], F32, tag="ps")
                for kt in range(NKH):
                    nc.tensor.matmul(pr[:], t_wr[:, kt, :], xn[kt][:, cs_own],
                                     start=(kt == 0), stop=(kt == NKH - 1))
                nc.scalar.activation(hw_sig[:, 512 * c:512 * (c + 1)], pr[:],
                                     AF.Sigmoid, bias=t_rb[:, 0:1])
            maskb = smalls.tile([128, W], BF16, tag="maskb")
            for c0, cn in ((0, 512), (512, 512), (1024, 192)):
                pb = psp.tile([128, 512], F32, tag="ps")
                nc.tensor.matmul(pb[:, :cn], ones_b[:], t_mask[:, c0:c0 + cn],
                                 start=True, stop=True)
                nc.scalar.activation(maskb[:, c0:c0 + cn], pb[:, :cn], AF.Copy)

            # conv-stack input tiles + interleaved conv-unit emitter
            hstES = ExitStack()
            hstp = hstES.enter_context(tc.tile_pool(name="hst", bufs=NKH))
            caccES = ExitStack()
            caccp = caccES.enter_context(tc.tile_pool(name="cacc", bufs=3))
            cbiasp = caccES.enter_context(tc.tile_pool(name="cbias", bufs=2))
            cgelp = caccES.enter_context(tc.tile_pool(name="cgel", bufs=1))
            hs = []
            for t in range(NKH):
                ht = hstp.tile([128, W], BF16, tag="h")
                nc.vector.tensor_scalar_mul(ht[:], xn[t][:, CTX - W:],
                                            t_nw1[:, t:t + 1])
                hs.append(ht)

            # DVE work units: conv-stack convs, head convs, head outputs.
            # Interleaved into the FFN emission so the scheduler overlaps
            # them with FFN matmuls instead of serializing at the tail.
            def stack_unit(t, j):
                d = CONV_DILS[j]
                bt = cbiasp.tile([128, W], BF16, tag="cbias")
                nc.vector.tensor_scalar_mul(bt[:], maskb[:], t_cb[:, t, j:j + 1])
                acc = caccp.tile([128, W], BF16, tag="cacc")
                nc.vector.scalar_tensor_tensor(acc[:], hs[t][:],
                                               t_cw[:, t, j, 3:4], bt[:],
                                               OP.mult, OP.add)
                for k in (2, 1, 0):
                    delta = (3 - k) * d
                    nc.vector.scalar_tensor_tensor(
                        acc[:, delta:], hs[t][:, :W - delta],
                        t_cw[:, t, j, k:k + 1], acc[:, delta:],
                        OP.mult, OP.add)
                ge = cgelp.tile([128, W], BF16, tag="cgel")
                nc.scalar.activation(ge[:], acc[:], AF.Gelu)
                nc.vector.tensor_add(hs[t][:], hs[t][:], ge[:])

            def head_unit(i, j):
                d = HEAD_DILS[i][j]
                wh = HEAD_WIN[i]
                ws0 = CTX - wh
                acc = haccp.tile([128, CTX], BF16, tag="hacc")
                nc.vector.tensor_scalar_mul(acc[:, :wh], xg[i][:, ws0:],
                                            t_hww[:, i, j, 3:4])
                for k in (2, 1, 0):
                    delta = (3 - k) * d
                    if delta < wh:
                        nc.vector.scalar_tensor_tensor(
                            acc[:, delta:wh], xg[i][:, ws0:CTX - delta],
                            t_hww[:, i, j, k:k + 1], acc[:, delta:wh],
                            OP.mult, OP.add)
                nc.vector.tensor_add(xg[i][:, ws0:], xg[i][:, ws0:], acc[:, :wh])

            def headout_unit(i):
                cp = cpfp.tile([128, OWN], BF16, tag="cpf")
                dma(cp[:], cprof_d[i])
                nc.vector.tensor_add(xg[i][:, OWN:], xg[i][:, OWN:], cp[:])
                for c in range(2):
                    cs = slice(512 * c, 512 * (c + 1))
                    pb = psp.tile([128, 512], F32, tag="ps")
                    nc.tensor.matmul(pb[:], t_oneh[:, 128 * i:128 * (i + 1)],
                                     hw_sig[:, cs], start=True, stop=True)
                    hb = sgp.tile([128, 512], BF16, tag="sg")
                    nc.scalar.activation(hb[:], pb[:], AF.Copy)
                    cso = slice(OWN + 512 * c, OWN + 512 * (c + 1))
                    nc.vector.tensor_mul(xg[i][:, cso], xg[i][:, cso], hb[:])

            units = []
            hq = [(i, j) for i in range(NH) for j in range(4)]  # j==3 -> out
            sq_ = [(t, j) for t in range(NKH) for j in range(6)]
            hi = si = 0
            while hi < len(hq) or si < len(sq_):
                if hi < len(hq):
                    units.append(("h", hq[hi])); hi += 1
                if si < len(sq_):
                    units.append(("s", sq_[si])); si += 1
            unit_iter = iter(units)

            def emit_conv_unit():
                try:
                    kind, arg = next(unit_iter)
                except StopIteration:
                    return False
                if kind == "s":
                    stack_unit(*arg)
                elif arg[1] == 3:
                    headout_unit(arg[0])
                else:
                    head_unit(*arg)
                return True

            # =========== Phase F: GLU FFN (own tokens) ===========
            ffnES = ExitStack()
            hfop = ffnES.enter_context(tc.tile_pool(name="hfo", bufs=4))
            wfopp = ffnES.enter_context(tc.tile_pool(name="wfop", bufs=4))
            hrpp = ffnES.enter_context(tc.tile_pool(name="hrp", bufs=6))
            headES = ExitStack()
            haccp = headES.enter_context(tc.tile_pool(name="hacc", bufs=1))
            cpfp = headES.enter_context(tc.tile_pool(name="cpf", bufs=2))

            hffd = drp.tile([2, NKI, 128, 512], BF16, tag="hffd")
            for p in range(NKI):
                wcs = wsl.tile([128, NKH, 128], BF16, tag="wsl")
                dma(wcs[:], wfi_d[p])
                wgs = wsl.tile([128, NKH, 128], BF16, tag="wsl")
                dma(wgs[:], wfi_d[p + NKI])
                for c in range(2):
                    cs_own = slice(OWN + 512 * c, OWN + 512 * (c + 1))
                    pc = psp.tile([128, 512], F32, tag="ps")
                    for kt in range(NKH):
                        nc.tensor.matmul(pc[:], wcs[:, kt, :], xn[kt][:, cs_own],
                                         start=(kt == 0), stop=(kt == NKH - 1))
                    pg = psp.tile([128, 512], F32, tag="ps")
                    for kt in range(NKH):
                        nc.tensor.matmul(pg[:], wgs[:, kt, :], xn[kt][:, cs_own],
                                         start=(kt == 0), stop=(kt == NKH - 1))
                    pcb = sgp.tile([128, 512], BF16, tag="pcb")
                    nc.scalar.activation(pcb[:], pc[:], AF.Copy)
                    sg = sgp.tile([128, 512], BF16, tag="sg")
                    nc.scalar.activation(sg[:], pg[:], AF.Sigmoid)
                    hoc = hfop.tile([128, 512], BF16, tag="hfo")
                    nc.vector.tensor_mul(hoc[:], pcb[:], sg[:])
                    dma(hffd[c, p], hoc[:])
                emit_conv_unit()
                emit_conv_unit()
                if p % 2 == 0:
                    emit_conv_unit()

            # ffn_out: 2 groups of 6 output tiles, streaming hff + weights
            for g in range(2):
                for c in range(2):
                    pss = []
                    for _j in range(6):
                        psj = psp.tile([128, 512], F32, tag="ps")
                        pss.append(psj)
                    for kt in range(NKI):
                        ws = wfopp.tile([128, 6, 128], BF16, tag="wfop")
                        dma(ws[:], wfo_d[g, kt])
                        hr = hrpp.tile([128, 512], BF16, tag="hr")
                        dma(hr[:], hffd[c, kt])
                        for j in range(6):
                            nc.tensor.matmul(pss[j][:], ws[:, j, :], hr[:],
                                             start=(kt == 0), stop=(kt == NKI - 1),
                                             skip_group_check=True)
                        if kt % 8 == 7:
                            emit_conv_unit()
                    for j in range(6):
                        pfb = sgp.tile([128, 512], BF16, tag="pcb")
                        nc.scalar.activation(pfb[:], pss[j][:], AF.Copy)
                        tf = ftp.tile([128, 512], BF16, tag="ftmp")
                        nc.vector.tensor_mul(tf[:], pfb[:],
                                             gfb[:, 512 * c:512 * (c + 1)])
                        dma(po_f[g * 6 + j, c], tf[:])
            while emit_conv_unit():
                pass
            headES.close()
            ffnES.close()
            caccES.close()

            # =========== conv-stack projection ===========
            for t in range(NKH):
                wp = wsl.tile([128, NKH, 128], BF16, tag="wsl")
                dma(wp[:], wproj_d[t])
                for c in range(2):
                    ws_ = slice(WPAD + 512 * c, WPAD + 512 * (c + 1))
                    pp = psp.tile([128, 512], F32, tag="ps")
                    for kt in range(NKH):
                        nc.tensor.matmul(pp[:], wp[:, kt, :], hs[kt][:, ws_],
                                         start=(kt == 0), stop=(kt == NKH - 1))
                    tb = sgp.tile([128, 512], BF16, tag="sg")
                    nc.scalar.activation(tb[:], pp[:], AF.Identity,
                                         bias=t_projb[:, t:t + 1])
                    tf = ftp.tile([128, 512], BF16, tag="ftmp")
                    nc.vector.tensor_mul(tf[:], tb[:],
                                         gcb[:, 512 * c:512 * (c + 1)])
                    dma(po_c[t, c], tf[:])
            hstES.close()

            # =========== Phase M: mix gate + mixing ===========
            sgmES = ExitStack()
            sgmp = sgmES.enter_context(tc.tile_pool(name="sgm", bufs=NH))
            sgm = []
            for t in range(NKH):
                wm = wsl.tile([128, NKH, 128], BF16, tag="wsl")
                dma(wm[:], wmg_d[t])
                st = sgmp.tile([128, OWN], BF16, tag="sgm")
                for c in range(2):
                    cs_own = slice(OWN + 512 * c, OWN + 512 * (c + 1))
                    pm = psp.tile([128, 512], F32, tag="ps")
                    for kt in range(NKH):
                        nc.tensor.matmul(pm[:], wm[:, kt, :],
                                         xg[kt][:, cs_own],
                                         start=(kt == 0), stop=(kt == NKH - 1))
                    nc.scalar.activation(st[:, 512 * c:512 * (c + 1)], pm[:],
                                         AF.Sigmoid, bias=t_mgb[:, t:t + 1])
                sgm.append(st)
            for t in range(NKH):
                nc.vector.tensor_mul(xg[t][:, OWN:], xg[t][:, OWN:], sgm[t][:])
            for t in range(NKH):
                wx = wsl.tile([128, NKH, 128], BF16, tag="wsl")
                dma(wx[:], wmix_d[t])
                for c in range(2):
                    cs_own = slice(OWN + 512 * c, OWN + 512 * (c + 1))
                    pm = psp.tile([128, 512], F32, tag="ps")
                    for kt in range(NKH):
                        nc.tensor.matmul(pm[:], wx[:, kt, :],
                                         xg[kt][:, cs_own],
                                         start=(kt == 0), stop=(kt == NKH - 1))
                    tb = sgp.tile([128, 512], BF16, tag="sg")
                    nc.scalar.activation(tb[:], pm[:], AF.Identity,
                                         bias=t_mixb[:, t:t + 1])
                    tf = ftp.tile([128, 512], BF16, tag="ftmp")
                    nc.vector.tensor_mul(tf[:], tb[:],
                                         gsb[:, 512 * c:512 * (c + 1)])
                    dma(po_s[t, c], tf[:])
            sgmES.close()
            xgES.close()

    nc.finalize()
    _fix_sync_capacity(nc, dummy[:])
    return nc


# ---------------------------------------------------------------------------
# host side
# ---------------------------------------------------------------------------
def _wslab(Wt, nk, no):
    """[IN, OUT] weight (already transposed to in-major) -> [no, 128, nk, 128]
    slab layout: slab[ot][p, kt, m] = Wt[kt*128+p, ot*128+m]."""
    return np.ascontiguousarray(
        Wt.reshape(nk, 128, no, 128).transpose(2, 1, 0, 3)
    ).astype(BF)


def _head_bias_profile(head_ws, head_bs):
    """Data-independent bias part of each head's (linear) conv chain over the
    global sequence, with exact causal zero padding."""
    C = np.zeros((NH, HD, S), np.float32)
    for i in range(NH):
        v = np.zeros((HD, S), np.float32)
        for j, d in enumerate(HEAD_DILS[i]):
            conv = np.zeros_like(v)
            for k in range(KK):
                delta = (3 - k) * d
                if delta == 0:
                    conv += head_ws[i, j, :, 0, k][:, None] * v
                elif delta < S:
                    conv[:, delta:] += head_ws[i, j, :, 0, k][:, None] * v[:, :-delta]
            v = v + conv + head_bs[i, j][:, None]
        C[i] = v
    return C


_NC_CACHE = {}


def kernel(**inputs):
    x = np.asarray(inputs["x"], np.float32)
    nw = np.asarray(inputs["norm_w"], np.float32)
    conv_ws = np.asarray(inputs["conv_ws"], np.float32)
    conv_bs = np.asarray(inputs["conv_bs"], np.float32)
    conv_proj_w = np.asarray(inputs["conv_proj_w"], np.float32)
    conv_proj_b = np.asarray(inputs["conv_proj_b"], np.float32)
    gate_w = np.asarray(inputs["gate_w"], np.float32)
    router_w = np.asarray(inputs["router_w"], np.float32)
    router_b = np.asarray(inputs["router_b"], np.float32)
    head_ws = np.asarray(inputs["head_ws"], np.float32)
    head_bs = np.asarray(inputs["head_bs"], np.float32)
    mix_gate_w = np.asarray(inputs["mix_gate_w"], np.float32)
    mix_gate_b = np.asarray(inputs["mix_gate_b"], np.float32)
    mixing_w = np.asarray(inputs["mixing_w"], np.float32)
    mixing_b = np.asarray(inputs["mixing_b"], np.float32)
    ffn_in_w = np.asarray(inputs["ffn_in_w"], np.float32)
    ffn_out_w = np.asarray(inputs["ffn_out_w"], np.float32)
    pg_w = np.asarray(inputs["pg_w"], np.float32)
    pg_b = np.asarray(inputs["pg_b"], np.float32)

    shared = {
        "cw": np.ascontiguousarray(
            conv_ws[:, :, 0, :].reshape(6, NKH, 128, KK).transpose(2, 1, 0, 3)),
        "cb": np.ascontiguousarray(
            conv_bs.reshape(6, NKH, 128).transpose(2, 1, 0)),
        "nw1": np.ascontiguousarray(nw[0].reshape(NKH, 128).T),
        "hww": np.ascontiguousarray(
            head_ws[:, :, :, 0, :].transpose(2, 0, 1, 3)),
        "wg": _wslab((gate_w * nw[1][None, :]).T, NKH, 24),
        "wr": np.ascontiguousarray(
            (router_w * nw[1][None, :]).T.reshape(NKH, 128, NH)
            .transpose(1, 0, 2)).astype(BF),
        "rb": router_b[:, None].astype(np.float32),
        "wpg": np.ascontiguousarray(
            (pg_w * nw).T.reshape(NKH, 128, 3).transpose(1, 0, 2)).astype(BF),
        "pgb": pg_b[:, None].astype(np.float32),
        "wproj": _wslab(conv_proj_w.T, NKH, NKH),
        "projb": np.ascontiguousarray(conv_proj_b.reshape(NKH, 128).T),
        "wmg": _wslab(mix_gate_w.T, NKH, NKH),
        "mgb": np.ascontiguousarray(mix_gate_b.reshape(NKH, 128).T),
        "wmix": _wslab(mixing_w.T, NKH, NKH),
        "mixb": np.ascontiguousarray(mixing_b.reshape(NKH, 128).T),
        "wfi": _wslab((ffn_in_w * nw[2][None, :]).T, NKH, 96),
        "wfo": np.ascontiguousarray(
            ffn_out_w.T.reshape(NKI, 128, 2, 6, 128).transpose(2, 0, 1, 3, 4)
        ).astype(BF),
    }
    oneh = np.zeros((NH, NH * 128), np.float32)
    for i in range(NH):
        oneh[i, 128 * i:128 * (i + 1)] = 1.0
    shared["oneh"] = oneh.astype(BF)

    cprof = _head_bias_profile(head_ws, head_bs)  # [NH, HD, S]
    cprof_h = [
        np.ascontiguousarray(cprof[:, :, h * OWN:(h + 1) * OWN]).astype(BF)
        for h in range(2)
    ]
    mask_h = []
    m0 = np.zeros((1, W), np.float32)
    m0[:, WPAD:] = 1.0
    mask_h.append(m0.astype(BF))
    mask_h.append(np.ones((1, W), BF))

    in_maps = []
    for core in range(N_CORES):
        b, h = core // 2, core % 2
        if h == 0:
            ctx = np.concatenate(
                [np.zeros((OWN, H), np.float32), x[b, :OWN]], axis=0)
        else:
            ctx = x[b]
        xc = np.ascontiguousarray(ctx.T.reshape(NKH, 128, CTX)).astype(BF)
        m = dict(shared)
        m["xc"] = xc
        m["cprof"] = cprof_h[h]
        m["mask"] = mask_h[h]
        in_maps.append(m)

    key = "nc"
    if key not in _NC_CACHE:
        _NC_CACHE[key] = _build()
    nc = _NC_CACHE[key]

    import os
    trace = bool(os.environ.get("BASS_KERNEL_TRACE"))
    r = run_bass_kernel_spmd(nc, in_maps, list(range(N_CORES)), trace=trace)
    global LAST_EXEC_NS
    LAST_EXEC_NS = r.exec_time_ns
    res = r.results

    out = np.empty((B, S, H), np.float32)
    for core in range(N_CORES):
        b, h = core // 2, core % 2
        total = np.zeros((H, OWN), np.float32)
        for name in ("po_c", "po_s", "po_f"):
            arr = np.asarray(res[core][name]).astype(np.float32)
            total += arr.transpose(0, 2, 1, 3).reshape(H, OWN)
        rows = slice(h * OWN, (h + 1) * OWN)
        out[b, rows, :] = x[b, rows, :] + total.T
    return out


# revision 13
# speedup vs baseline: 1.1540x; 1.0292x over previous
"""Trainium2 Bass kernel for nn_DCLMBlock (B=4, S=2048, H=1536) on 8 NeuronCores.

Sharding: token-parallel. Core c handles batch b=c//2, sequence half h=c%2
(1024 tokens). Every core receives a causal *context* of 2048 tokens ending at
its last own token (zero-padded in front for first halves), so one SPMD
program serves all cores:
  - gate matmul + the 12 head conv chains run over the full context (head
    dilations reach back up to 6144 tokens),
  - the dilated-conv stack runs over own tokens + a 192-token halo
    (cumulative receptive field of the 6 convs = 189),
  - everything else (router, mix, proj, FFN, path gates) over own 1024 tokens.

All activations live feature-major ([feature_partition, token_free]); matmuls
run in bf16 with fp32 PSUM accumulation; depthwise dilated convs are fused
DVE scalar_tensor_tensor chains (out = x_shift*w_tap + acc) with clamped tap
ranges that reproduce exact causal zero-padding. Head-conv biases are handled
by linearity: the on-chip chain is bias-free and the host adds a precomputed
data-independent bias profile. The three path outputs are returned separately
(scaled by their sigmoid path gates) and the host does the final fp32
residual sum out = x + pc + ps + pf.
"""

import numpy as np
import ml_dtypes

import concourse.bass as bass
import concourse.tile as tile
from concourse import mybir
from concourse.bass_utils import run_bass_kernel_spmd

F32 = mybir.dt.float32
BF16 = mybir.dt.bfloat16
AF = mybir.ActivationFunctionType
OP = mybir.AluOpType
BF = ml_dtypes.bfloat16

B, S, H = 4, 2048, 1536
NH, HD = 12, 128
KK = 4
INNER = 6144
EPS = 1e-6
CONV_DILS = (1, 2, 4, 8, 16, 32)
HEAD_DILS = [(1, 2, 4), (1, 1, 1), (4, 8, 16), (8, 16, 32), (32, 64, 128),
             (64, 128, 256), (256, 512, 1024), (1, 100, 200), (1, 500, 1000),
             (1, 1024, 2048), (3, 9, 27), (5, 25, 125)]

HEAD_HALO = [3 * sum(ds) for ds in HEAD_DILS]
NKH = H // 128          # 12 feature tiles
NKI = INNER // 128      # 48 inner tiles
CTX = 2048              # context tokens per core
OWN = 1024              # own tokens per core
W = 1216                # conv-stack window (192 halo + 1024 own)
HEAD_WIN = [min(2048, (1024 + h + 1) // 2 * 2) for h in HEAD_HALO]
HEAD_CSTART = [max(0, (2048 - w) // 512) for w in HEAD_WIN]
WPAD = 192
N_CORES = 8


# ---------------------------------------------------------------------------
# walrus sync-wait capacity workaround (this build allows <=1 wait per inst)
# ---------------------------------------------------------------------------
def _fix_sync_capacity(nc, dummy_ap):
    ET = mybir.EngineType
    was_frozen = nc._frozen
    nc._frozen = False

    def mk_stub(engine_ty, waits, updates):
        if engine_ty == ET.SP:
            inst = nc.sync.nop(nofuse=True)
        elif engine_ty == ET.DVE:
            inst = nc.vector.tensor_copy(dummy_ap, dummy_ap)
        elif engine_ty == ET.Activation:
            inst = nc.scalar.activation(dummy_ap, dummy_ap, AF.Copy)
        elif engine_ty == ET.PE:
            inst = nc.tensor.drain()
        elif engine_ty == ET.Pool:
            inst = nc.gpsimd.tensor_copy(dummy_ap, dummy_ap)
        else:
            raise RuntimeError(f"no stub for engine {engine_ty}")
        popped = nc.cur_bb.bb.instructions.pop()
        assert popped is inst.ins
        inst.ins.sync_info = mybir.SyncInfo(
            on_wait=list(waits), on_update=list(updates)
        )
        return inst.ins

    for f in nc.m.functions:
        for bb in f.blocks:
            out = []
            changed = False
            for ins in bb.instructions:
                si = ins.sync_info
                if si is not None and len(si.on_wait) > 1:
                    for wt in si.on_wait[1:]:
                        out.append(mk_stub(ins.engine, [wt], []))
                    del si.on_wait[1:]
                    changed = True
                out.append(ins)
                if si is not None and len(si.on_update) > 1:
                    for u in si.on_update[1:]:
                        out.append(mk_stub(ins.engine, [], [u]))
                    del si.on_update[1:]
                    changed = True
            if changed:
                bb.instructions[:] = out
    nc._frozen = was_frozen


# ---------------------------------------------------------------------------
# device program
# ---------------------------------------------------------------------------
def _build():
    nc = bass.Bass()
    dummy = nc.sbuf_tensor([1, 1], F32).__enter__()

    def par(name, shape, dt):
        return nc.declare_dram_parameter(name, shape, dt, isOutput=False)

    xc_d = par("xc", [NKH, 128, CTX], BF16)
    cw_d = par("cw", [128, NKH, 6, KK], F32)
    cb_d = par("cb", [128, NKH, 6], F32)
    nw1_d = par("nw1", [128, NKH], F32)
    hww_d = par("hww", [128, NH, 3, KK], F32)
    cprof_d = par("cprof", [NH, 128, OWN], BF16)
    mask_d = par("mask", [1, W], BF16)
    oneh_d = par("oneh", [NH, NH * 128], BF16)
    wg_d = par("wg", [24, 128, NKH, 128], BF16)
    wr_d = par("wr", [128, NKH, NH], BF16)
    rb_d = par("rb", [NH, 1], F32)
    wpg_d = par("wpg", [128, NKH, 3], BF16)
    pgb_d = par("pgb", [3, 1], F32)
    wproj_d = par("wproj", [NKH, 128, NKH, 128], BF16)
    projb_d = par("projb", [128, NKH], F32)
    wmg_d = par("wmg", [NKH, 128, NKH, 128], BF16)
    mgb_d = par("mgb", [128, NKH], F32)
    wmix_d = par("wmix", [NKH, 128, NKH, 128], BF16)
    mixb_d = par("mixb", [128, NKH], F32)
    wfi_d = par("wfi", [96, 128, NKH, 128], BF16)
    wfo_d = par("wfo", [2, NKI, 128, 6, 128], BF16)

    po_c = nc.declare_dram_parameter("po_c", [NKH, 2, 128, 512], BF16, isOutput=True)
    po_s = nc.declare_dram_parameter("po_s", [NKH, 2, 128, 512], BF16, isOutput=True)
    po_f = nc.declare_dram_parameter("po_f", [NKH, 2, 128, 512], BF16, isOutput=True)

    dma = nc.sync.dma_start

    from contextlib import ExitStack
    with tile.TileContext(nc) as tc:
        with ExitStack() as es:
            ec = es.enter_context
            const = ec(tc.tile_pool(name="const", bufs=1))
            psp = ec(tc.tile_pool(name="ps", bufs=8, space="PSUM"))
            xnp = ec(tc.tile_pool(name="xn", bufs=NKH))
            wsl = ec(tc.tile_pool(name="wsl", bufs=4))
            smalls = ec(tc.tile_pool(name="smalls", bufs=1))
            g3bp = ec(tc.tile_pool(name="g3bp", bufs=3))
            sgp = ec(tc.tile_pool(name="sgt", bufs=3))
            ftp = ec(tc.tile_pool(name="ftmp", bufs=4))
            drp = ec(tc.tile_pool(name="dram", bufs=1, space="DRAM"))

            # ---- constants ----
            t_cw = const.tile([128, NKH, 6, KK], F32)
            dma(t_cw[:], cw_d[:])
            t_cb = const.tile([128, NKH, 6], F32)
            dma(t_cb[:], cb_d[:])
            t_nw1 = const.tile([128, NKH], F32)
            dma(t_nw1[:], nw1_d[:])
            t_hww = const.tile([128, NH, 3, KK], F32)
            dma(t_hww[:], hww_d[:])
            t_mask = const.tile([1, W], BF16)
            dma(t_mask[:], mask_d[:])
            t_oneh = const.tile([NH, NH * 128], BF16)
            dma(t_oneh[:], oneh_d[:])
            t_wr = const.tile([128, NKH, NH], BF16)
            dma(t_wr[:], wr_d[:])
            t_rb = const.tile([NH, 1], F32)
            dma(t_rb[:], rb_d[:])
            t_wpg = const.tile([128, NKH, 3], BF16)
            dma(t_wpg[:], wpg_d[:])
            t_pgb = const.tile([3, 1], F32)
            dma(t_pgb[:], pgb_d[:])
            t_projb = const.tile([128, NKH], F32)
            dma(t_projb[:], projb_d[:])
            t_mgb = const.tile([128, NKH], F32)
            dma(t_mgb[:], mgb_d[:])
            t_mixb = const.tile([128, NKH], F32)
            dma(t_mixb[:], mixb_d[:])
            ones_k = const.tile([128, 1], BF16)
            nc.vector.memset(ones_k[:], 1.0)
            eps_t = const.tile([1, 1], F32)
            nc.vector.memset(eps_t[:], EPS)
            ones_b = const.tile([1, 128], BF16)
            nc.vector.memset(ones_b[:], 1.0)

            # =========== Phase R: load x, rmsnorm scale, xn ===========
            esR = ExitStack()
            xcp = esR.enter_context(tc.tile_pool(name="xc", bufs=NKH))
            x2p = esR.enter_context(tc.tile_pool(name="x2", bufs=3))
            rtp = esR.enter_context(tc.tile_pool(name="rt", bufs=1))
            rbcp = esR.enter_context(tc.tile_pool(name="rbc", bufs=1))

            xct = []
            for t in range(NKH):
                xt = xcp.tile([128, CTX], BF16, tag="xc")
                dma(xt[:], xc_d[t])
                xct.append(xt)
            sq = rtp.tile([1, CTX], F32, tag="sq")
            for c in range(4):
                cs = slice(512 * c, 512 * (c + 1))
                pss = psp.tile([1, 512], F32, tag="ps")
                for t in range(NKH):
                    x2t = x2p.tile([128, 512], BF16, tag="x2")
                    nc.vector.tensor_mul(x2t[:], xct[t][:, cs], xct[t][:, cs])
                    nc.tensor.matmul(pss[:], ones_k[:], x2t[:],
                                     start=(t == 0), stop=(t == NKH - 1))
                nc.scalar.activation(sq[:, cs], pss[:], AF.Sqrt,
                                     bias=eps_t[:], scale=1.0 / H)
            dsc = drp.tile([CTX], F32, tag="dsc")
            dma(dsc.rearrange("(p f) -> p f", p=1), sq[:])
            r128 = rtp.tile([128, CTX // 128], F32, tag="r128")
            dma(r128[:], dsc.rearrange("(p f) -> p f", p=128))
            nc.vector.reciprocal(r128[:], r128[:])
            r128b = rtp.tile([128, CTX // 128], BF16, tag="r128b")
            nc.vector.tensor_copy(r128b[:], r128[:])
            dsc2 = drp.tile([CTX], BF16, tag="dsc2")
            dma(dsc2.rearrange("(p f) -> p f", p=128), r128b[:])
            r1 = rtp.tile([1, CTX], BF16, tag="r1")
            dma(r1[:], dsc2.rearrange("(p f) -> p f", p=1))
            rb128 = rbcp.tile([128, CTX], BF16)
            for c in range(4):
                cs = slice(512 * c, 512 * (c + 1))
                pb = psp.tile([128, 512], F32, tag="ps")
                nc.tensor.matmul(pb[:], ones_b[:], r1[:, cs],
                                 start=True, stop=True)
                nc.scalar.activation(rb128[:, cs], pb[:], AF.Copy)
            xn = []
            for t in range(NKH):
                xnt = xnp.tile([128, CTX], BF16, tag="xn")
                nc.vector.tensor_mul(xnt[:], xct[t][:], rb128[:])
                xn.append(xnt)
            esR.close()

            # =========== Phase G: gate matmul (full ctx) + x_gated ===========
            xgES = ExitStack()
            xgp = xgES.enter_context(tc.tile_pool(name="xg", bufs=NH))
            xg = []
            for t in range(NH):
                wc = wsl.tile([128, NKH, 128], BF16, tag="wsl")
                dma(wc[:], wg_d[t])
                wgt = wsl.tile([128, NKH, 128], BF16, tag="wsl")
                dma(wgt[:], wg_d[t + NH])
                xgt = xgp.tile([128, CTX], BF16, tag="xg")
                for c in range(HEAD_CSTART[t], 4):
                    cs = slice(512 * c, 512 * (c + 1))
                    pc = psp.tile([128, 512], F32, tag="ps")
                    for kt in range(NKH):
                        nc.tensor.matmul(pc[:], wc[:, kt, :], xn[kt][:, cs],
                                         start=(kt == 0), stop=(kt == NKH - 1))
                    pg = psp.tile([128, 512], F32, tag="ps")
                    for kt in range(NKH):
                        nc.tensor.matmul(pg[:], wgt[:, kt, :], xn[kt][:, cs],
                                         start=(kt == 0), stop=(kt == NKH - 1))
                    pcb = sgp.tile([128, 512], BF16, tag="pcb")
                    nc.scalar.activation(pcb[:], pc[:], AF.Copy)
                    sg = sgp.tile([128, 512], BF16, tag="sg")
                    nc.scalar.activation(sg[:], pg[:], AF.Sigmoid)
                    nc.vector.tensor_mul(xgt[:, cs], pcb[:], sg[:])
                xg.append(xgt)

            # =========== Phase P: path gates, router, mask broadcast =========
            pgs = smalls.tile([3, OWN], BF16, tag="pgs")
            for c in range(2):
                cs_own = slice(OWN + 512 * c, OWN + 512 * (c + 1))
                pp = psp.tile([3, 512], F32, tag="ps")
                for kt in range(NKH):
                    nc.tensor.matmul(pp[:], t_wpg[:, kt, :], xn[kt][:, cs_own],
                                     start=(kt == 0), stop=(kt == NKH - 1))
                nc.scalar.activation(pgs[:, 512 * c:512 * (c + 1)], pp[:],
                                     AF.Sigmoid, bias=t_pgb[:, 0:1])
            g3b = []
            for i in range(3):
                gt = g3bp.tile([128, OWN], BF16, tag="g3b")
                for c in range(2):
                    cs = slice(512 * c, 512 * (c + 1))
                    pb = psp.tile([128, 512], F32, tag="ps")
                    nc.tensor.matmul(pb[:], t_oneh[0:3, 128 * i:128 * (i + 1)],
                                     pgs[:, cs], start=True, stop=True)
                    nc.scalar.activation(gt[:, cs], pb[:], AF.Copy)
                g3b.append(gt)
            gcb, gsb, gfb = g3b
            hw_sig = smalls.tile([NH, OWN], BF16, tag="hw_sig")
            for c in range(2):
                cs_own = slice(OWN + 512 * c, OWN + 512 * (c + 1))
                pr = psp.tile([NH, 512], F32, tag="ps")
                for kt in range(NKH):
                    nc.tensor.matmul(pr[:], t_wr[:, kt, :], xn[kt][:, cs_own],
                                     start=(kt == 0), stop=(kt == NKH - 1))
                nc.scalar.activation(hw_sig[:, 512 * c:512 * (c + 1)], pr[:],
                                     AF.Sigmoid, bias=t_rb[:, 0:1])
            maskb = smalls.tile([128, W], BF16, tag="maskb")
            for c0, cn in ((0, 512), (512, 512), (1024, 192)):
                pb = psp.tile([128, 512], F32, tag="ps")
                nc.tensor.matmul(pb[:, :cn], ones_b[:], t_mask[:, c0:c0 + cn],
                                 start=True, stop=True)
                nc.scalar.activation(maskb[:, c0:c0 + cn], pb[:, :cn], AF.Copy)

            # conv-stack input tiles + interleaved conv-unit emitter
            hstES = ExitStack()
            hstp = hstES.enter_context(tc.tile_pool(name="hst", bufs=NKH))
            caccES = ExitStack()
            caccp = caccES.enter_context(tc.tile_pool(name="cacc", bufs=3))
            cbiasp = caccES.enter_context(tc.tile_pool(name="cbias", bufs=2))
            cgelp = caccES.enter_context(tc.tile_pool(name="cgel", bufs=1))
            hs = []
            for t in range(NKH):
                ht = hstp.tile([128, W], BF16, tag="h")
                nc.vector.tensor_scalar_mul(ht[:], xn[t][:, CTX - W:],
                                            t_nw1[:, t:t + 1])
                hs.append(ht)

            # DVE work units: conv-stack convs, head convs, head outputs.
            # Interleaved into the FFN emission so the scheduler overlaps
            # them with FFN matmuls instead of serializing at the tail.
            def stack_unit(t, j):
                d = CONV_DILS[j]
                bt = cbiasp.tile([128, W], BF16, tag="cbias")
                nc.vector.tensor_scalar_mul(bt[:], maskb[:], t_cb[:, t, j:j + 1])
                acc = caccp.tile([128, W], BF16, tag="cacc")
                nc.vector.scalar_tensor_tensor(acc[:], hs[t][:],
                                               t_cw[:, t, j, 3:4], bt[:],
                                               OP.mult, OP.add)
                for k in (2, 1, 0):
                    delta = (3 - k) * d
                    nc.vector.scalar_tensor_tensor(
                        acc[:, delta:], hs[t][:, :W - delta],
                        t_cw[:, t, j, k:k + 1], acc[:, delta:],
                        OP.mult, OP.add)
                ge = cgelp.tile([128, W], BF16, tag="cgel")
                nc.scalar.activation(ge[:], acc[:], AF.Gelu)
                nc.vector.tensor_add(hs[t][:], hs[t][:], ge[:])

            def head_unit(i, j):
                d = HEAD_DILS[i][j]
                wh = HEAD_WIN[i]
                ws0 = CTX - wh
                acc = haccp.tile([128, CTX], BF16, tag="hacc")
                nc.vector.tensor_scalar_mul(acc[:, :wh], xg[i][:, ws0:],
                                            t_hww[:, i, j, 3:4])
                for k in (2, 1, 0):
                    delta = (3 - k) * d
                    if delta < wh:
                        nc.vector.scalar_tensor_tensor(
                            acc[:, delta:wh], xg[i][:, ws0:CTX - delta],
                            t_hww[:, i, j, k:k + 1], acc[:, delta:wh],
                            OP.mult, OP.add)
                nc.vector.tensor_add(xg[i][:, ws0:], xg[i][:, ws0:], acc[:, :wh])

            def headout_unit(i):
                cp = cpfp.tile([128, OWN], BF16, tag="cpf")
                dma(cp[:], cprof_d[i])
                nc.vector.tensor_add(xg[i][:, OWN:], xg[i][:, OWN:], cp[:])
                for c in range(2):
                    cs = slice(512 * c, 512 * (c + 1))
                    pb = psp.tile([128, 512], F32, tag="ps")
                    nc.tensor.matmul(pb[:], t_oneh[:, 128 * i:128 * (i + 1)],
                                     hw_sig[:, cs], start=True, stop=True)
                    hb = sgp.tile([128, 512], BF16, tag="sg")
                    nc.scalar.activation(hb[:], pb[:], AF.Copy)
                    cso = slice(OWN + 512 * c, OWN + 512 * (c + 1))
                    nc.vector.tensor_mul(xg[i][:, cso], xg[i][:, cso], hb[:])

            units = []
            hq = [(i, j) for i in range(NH) for j in range(4)]  # j==3 -> out
            sq_ = [(t, j) for t in range(NKH) for j in range(6)]
            hi = si = 0
            while hi < len(hq) or si < len(sq_):
                if hi < len(hq):
                    units.append(("h", hq[hi])); hi += 1
                if si < len(sq_):
                    units.append(("s", sq_[si])); si += 1
            unit_iter = iter(units)

            def emit_conv_unit():
                try:
                    kind, arg = next(unit_iter)
                except StopIteration:
                    return False
                if kind == "s":
                    stack_unit(*arg)
                elif arg[1] == 3:
                    headout_unit(arg[0])
                else:
                    head_unit(*arg)
                return True

            # =========== Phase F: GLU FFN (own tokens) ===========
            ffnES = ExitStack()
            hfop = ffnES.enter_context(tc.tile_pool(name="hfo", bufs=4))
            wfopp = ffnES.enter_context(tc.tile_pool(name="wfop", bufs=4))
            hrpp = ffnES.enter_context(tc.tile_pool(name="hrp", bufs=6))
            headES = ExitStack()
            haccp = headES.enter_context(tc.tile_pool(name="hacc", bufs=1))
            cpfp = headES.enter_context(tc.tile_pool(name="cpf", bufs=2))

            hffd = drp.tile([2, NKI, 128, 512], BF16, tag="hffd")
            for p in range(NKI):
                wcs = wsl.tile([128, NKH, 128], BF16, tag="wsl")
                dma(wcs[:], wfi_d[p])
                wgs = wsl.tile([128, NKH, 128], BF16, tag="wsl")
                dma(wgs[:], wfi_d[p + NKI])
                for c in range(2):
                    cs_own = slice(OWN + 512 * c, OWN + 512 * (c + 1))
                    pc = psp.tile([128, 512], F32, tag="ps")
                    for kt in range(NKH):
                        nc.tensor.matmul(pc[:], wcs[:, kt, :], xn[kt][:, cs_own],
                                         start=(kt == 0), stop=(kt == NKH - 1))
                    pg = psp.tile([128, 512], F32, tag="ps")
                    for kt in range(NKH):
                        nc.tensor.matmul(pg[:], wgs[:, kt, :], xn[kt][:, cs_own],
                                         start=(kt == 0), stop=(kt == NKH - 1))
                    pcb = sgp.tile([128, 512], BF16, tag="pcb")
                    nc.scalar.activation(pcb[:], pc[:], AF.Copy)
                    sg = sgp.tile([128, 512], BF16, tag="sg")
                    nc.scalar.activation(sg[:], pg[:], AF.Sigmoid)
                    hoc = hfop.tile([128, 512], BF16, tag="hfo")
                    nc.vector.tensor_mul(hoc[:], pcb[:], sg[:])
                    dma(hffd[c, p], hoc[:])
                emit_conv_unit()
                emit_conv_unit()
                if p % 2 == 0:
                    emit_conv_unit()

            # ffn_out: 2 groups of 6 output tiles, streaming hff + weights
            for g in range(2):
                for c in range(2):
                    pss = []
                    for _j in range(6):
                        psj = psp.tile([128, 512], F32, tag="ps")
                        pss.append(psj)
                    for kt in range(NKI):
                        ws = wfopp.tile([128, 6, 128], BF16, tag="wfop")
                        dma(ws[:], wfo_d[g, kt])
                        hr = hrpp.tile([128, 512], BF16, tag="hr")
                        dma(hr[:], hffd[c, kt])
                        for j in range(6):
                            nc.tensor.matmul(pss[j][:], ws[:, j, :], hr[:],
                                             start=(kt == 0), stop=(kt == NKI - 1),
                                             skip_group_check=True)
                    for j in range(6):
                        pfb = sgp.tile([128, 512], BF16, tag="pcb")
                        nc.scalar.activation(pfb[:], pss[j][:], AF.Copy)
                        tf = ftp.tile([128, 512], BF16, tag="ftmp")
                        nc.vector.tensor_mul(tf[:], pfb[:],
                                             gfb[:, 512 * c:512 * (c + 1)])
                        dma(po_f[g * 6 + j, c], tf[:])
                    for _ in range(6):
                        emit_conv_unit()
            while emit_conv_unit():
                pass
            headES.close()
            ffnES.close()
            caccES.close()

            # =========== conv-stack projection ===========
            for t in range(NKH):
                wp = wsl.tile([128, NKH, 128], BF16, tag="wsl")
                dma(wp[:], wproj_d[t])
                for c in range(2):
                    ws_ = slice(WPAD + 512 * c, WPAD + 512 * (c + 1))
                    pp = psp.tile([128, 512], F32, tag="ps")
                    for kt in range(NKH):
                        nc.tensor.matmul(pp[:], wp[:, kt, :], hs[kt][:, ws_],
                                         start=(kt == 0), stop=(kt == NKH - 1))
                    tb = sgp.tile([128, 512], BF16, tag="sg")
                    nc.scalar.activation(tb[:], pp[:], AF.Identity,
                                         bias=t_projb[:, t:t + 1])
                    tf = ftp.tile([128, 512], BF16, tag="ftmp")
                    nc.vector.tensor_mul(tf[:], tb[:],
                                         gcb[:, 512 * c:512 * (c + 1)])
                    dma(po_c[t, c], tf[:])
            hstES.close()

            # =========== Phase M: mix gate + mixing ===========
            sgmES = ExitStack()
            sgmp = sgmES.enter_context(tc.tile_pool(name="sgm", bufs=NH))
            sgm = []
            for t in range(NKH):
                wm = wsl.tile([128, NKH, 128], BF16, tag="wsl")
                dma(wm[:], wmg_d[t])
                st = sgmp.tile([128, OWN], BF16, tag="sgm")
                for c in range(2):
                    cs_own = slice(OWN + 512 * c, OWN + 512 * (c + 1))
                    pm = psp.tile([128, 512], F32, tag="ps")
                    for kt in range(NKH):
                        nc.tensor.matmul(pm[:], wm[:, kt, :],
                                         xg[kt][:, cs_own],
                                         start=(kt == 0), stop=(kt == NKH - 1))
                    nc.scalar.activation(st[:, 512 * c:512 * (c + 1)], pm[:],
                                         AF.Sigmoid, bias=t_mgb[:, t:t + 1])
                sgm.append(st)
            for t in range(NKH):
                nc.vector.tensor_mul(xg[t][:, OWN:], xg[t][:, OWN:], sgm[t][:])
            for t in range(NKH):
                wx = wsl.tile([128, NKH, 128], BF16, tag="wsl")
                dma(wx[:], wmix_d[t])
                for c in range(2):
                    cs_own = slice(OWN + 512 * c, OWN + 512 * (c + 1))
                    pm = psp.tile([128, 512], F32, tag="ps")
                    for kt in range(NKH):
                        nc.tensor.matmul(pm[:], wx[:, kt, :],
                                         xg[kt][:, cs_own],
                                         start=(kt == 0), stop=(kt == NKH - 1))
                    tb = sgp.tile([128, 512], BF16, tag="sg")
                    nc.scalar.activation(tb[:], pm[:], AF.Identity,
                                         bias=t_mixb[:, t:t + 1])
                    tf = ftp.tile([128, 512], BF16, tag="ftmp")
                    nc.vector.tensor_mul(tf[:], tb[:],
                                         gsb[:, 512 * c:512 * (c + 1)])
                    dma(po_s[t, c], tf[:])
            sgmES.close()
            xgES.close()

    nc.finalize()
    _fix_sync_capacity(nc, dummy[:])
    return nc


# ---------------------------------------------------------------------------
# host side
# ---------------------------------------------------------------------------
def _wslab(Wt, nk, no):
    """[IN, OUT] weight (already transposed to in-major) -> [no, 128, nk, 128]
    slab layout: slab[ot][p, kt, m] = Wt[kt*128+p, ot*128+m]."""
    return np.ascontiguousarray(
        Wt.reshape(nk, 128, no, 128).transpose(2, 1, 0, 3)
    ).astype(BF)


def _head_bias_profile(head_ws, head_bs):
    """Data-independent bias part of each head's (linear) conv chain over the
    global sequence, with exact causal zero padding."""
    C = np.zeros((NH, HD, S), np.float32)
    for i in range(NH):
        v = np.zeros((HD, S), np.float32)
        for j, d in enumerate(HEAD_DILS[i]):
            conv = np.zeros_like(v)
            for k in range(KK):
                delta = (3 - k) * d
                if delta == 0:
                    conv += head_ws[i, j, :, 0, k][:, None] * v
                elif delta < S:
                    conv[:, delta:] += head_ws[i, j, :, 0, k][:, None] * v[:, :-delta]
            v = v + conv + head_bs[i, j][:, None]
        C[i] = v
    return C


_NC_CACHE = {}


def kernel(**inputs):
    x = np.asarray(inputs["x"], np.float32)
    nw = np.asarray(inputs["norm_w"], np.float32)
    conv_ws = np.asarray(inputs["conv_ws"], np.float32)
    conv_bs = np.asarray(inputs["conv_bs"], np.float32)
    conv_proj_w = np.asarray(inputs["conv_proj_w"], np.float32)
    conv_proj_b = np.asarray(inputs["conv_proj_b"], np.float32)
    gate_w = np.asarray(inputs["gate_w"], np.float32)
    router_w = np.asarray(inputs["router_w"], np.float32)
    router_b = np.asarray(inputs["router_b"], np.float32)
    head_ws = np.asarray(inputs["head_ws"], np.float32)
    head_bs = np.asarray(inputs["head_bs"], np.float32)
    mix_gate_w = np.asarray(inputs["mix_gate_w"], np.float32)
    mix_gate_b = np.asarray(inputs["mix_gate_b"], np.float32)
    mixing_w = np.asarray(inputs["mixing_w"], np.float32)
    mixing_b = np.asarray(inputs["mixing_b"], np.float32)
    ffn_in_w = np.asarray(inputs["ffn_in_w"], np.float32)
    ffn_out_w = np.asarray(inputs["ffn_out_w"], np.float32)
    pg_w = np.asarray(inputs["pg_w"], np.float32)
    pg_b = np.asarray(inputs["pg_b"], np.float32)

    shared = {
        "cw": np.ascontiguousarray(
            conv_ws[:, :, 0, :].reshape(6, NKH, 128, KK).transpose(2, 1, 0, 3)),
        "cb": np.ascontiguousarray(
            conv_bs.reshape(6, NKH, 128).transpose(2, 1, 0)),
        "nw1": np.ascontiguousarray(nw[0].reshape(NKH, 128).T),
        "hww": np.ascontiguousarray(
            head_ws[:, :, :, 0, :].transpose(2, 0, 1, 3)),
        "wg": _wslab((gate_w * nw[1][None, :]).T, NKH, 24),
        "wr": np.ascontiguousarray(
            (router_w * nw[1][None, :]).T.reshape(NKH, 128, NH)
            .transpose(1, 0, 2)).astype(BF),
        "rb": router_b[:, None].astype(np.float32),
        "wpg": np.ascontiguousarray(
            (pg_w * nw).T.reshape(NKH, 128, 3).transpose(1, 0, 2)).astype(BF),
        "pgb": pg_b[:, None].astype(np.float32),
        "wproj": _wslab(conv_proj_w.T, NKH, NKH),
        "projb": np.ascontiguousarray(conv_proj_b.reshape(NKH, 128).T),
        "wmg": _wslab(mix_gate_w.T, NKH, NKH),
        "mgb": np.ascontiguousarray(mix_gate_b.reshape(NKH, 128).T),
        "wmix": _wslab(mixing_w.T, NKH, NKH),
        "mixb": np.ascontiguousarray(mixing_b.reshape(NKH, 128).T),
        "wfi": _wslab((ffn_in_w * nw[2][None, :]).T, NKH, 96),
        "wfo": np.ascontiguousarray(
            ffn_out_w.T.reshape(NKI, 128, 2, 6, 128).transpose(2, 0, 1, 3, 4)
        ).astype(BF),
    }
    oneh = np.zeros((NH, NH * 128), np.float32)
    for i in range(NH):
        oneh[i, 128 * i:128 * (i + 1)] = 1.0
    shared["oneh"] = oneh.astype(BF)

    cprof = _head_bias_profile(head_ws, head_bs)  # [NH, HD, S]
    cprof_h = [
        np.ascontiguousarray(cprof[:, :, h * OWN:(h + 1) * OWN]).astype(BF)
        for h in range(2)
    ]
    mask_h = []
    m0 = np.zeros((1, W), np.float32)
    m0[:, WPAD:] = 1.0
    mask_h.append(m0.astype(BF))
    mask_h.append(np.ones((1, W), BF))

    in_maps = []
    for core in range(N_CORES):
        b, h = core // 2, core % 2
        if h == 0:
            ctx = np.concatenate(
                [np.zeros((OWN, H), np.float32), x[b, :OWN]], axis=0)
        else:
            ctx = x[b]
        xc = np.ascontiguousarray(ctx.T.reshape(NKH, 128, CTX)).astype(BF)
        m = dict(shared)
        m["xc"] = xc
        m["cprof"] = cprof_h[h]
        m["mask"] = mask_h[h]
        in_maps.append(m)

    key = "nc"
    if key not in _NC_CACHE:
        _NC_CACHE[key] = _build()
    nc = _NC_CACHE[key]

    import os
    trace = bool(os.environ.get("BASS_KERNEL_TRACE"))
    r = run_bass_kernel_spmd(nc, in_maps, list(range(N_CORES)), trace=trace)
    global LAST_EXEC_NS
    LAST_EXEC_NS = r.exec_time_ns
    res = r.results

    out = np.empty((B, S, H), np.float32)
    for core in range(N_CORES):
        b, h = core // 2, core % 2
        total = np.zeros((H, OWN), np.float32)
        for name in ("po_c", "po_s", "po_f"):
            arr = np.asarray(res[core][name]).astype(np.float32)
            total += arr.transpose(0, 2, 1, 3).reshape(H, OWN)
        rows = slice(h * OWN, (h + 1) * OWN)
        out[b, rows, :] = x[b, rows, :] + total.T
    return out
